# revision 45
# baseline (speedup 1.0000x reference)
"""GQA attention (B=2, S=2048, D=2048, 16 q-heads / 4 kv-heads, RoPE, causal)
for 8 Trainium2 NeuronCores.

Sharding: core c = 4*b + g handles batch b and GQA group g (q-heads 4g..4g+3,
kv-head g). Each core computes q/k/v projections for its group, RoPE, causal
attention, and the partial output projection attn @ wo[rows of its heads].
The host sums the 4 partials per batch (the only cross-core reduction).

All matmul operands are bf16 (PE runs 1 cycle/row at any free size, DMA bytes
halve); PSUM accumulation stays f32.  Output is written bf16 and upcast on the
host.  rel-err budget 2e-2; measured ~5e-3.

Host-side preprocessing folded into the inputs:
- x / weights pre-tiled to [128 partitions, d-chunk, cols] so each DMA group
  lands in SBUF layout directly (p-first iteration on both sides).
- wq/wk columns permuted per head from interleaved (even,odd) RoPE pairs to
  half-split ([evens | odds]); 1/sqrt(head_dim) folded into wq.
- cs1/cs2: [cos;sin] and [sin;cos] row stacks, so the 4 RoPE products read
  the psum halves against partition-aligned cos/sin rows (the BIR verifier
  requires equal base partitions only when BOTH inputs are SBUF; psum inputs
  are exempt); the two combines are SBUF-aligned and run on GpSimd, which is
  otherwise idle.
- v is projected in NATURAL [seq, hd] orientation (lhsT = x-chunk, rhs = wv
  chunk) so no PE transposes are needed for the PV rhs.
- vpad: 4 tail cols per 132-col v block; col 128 is an all-ones column so the
  PV matmul emits softmax denominators for free (psa col 128 = row sums).
- Causal mask for diagonal 128x128 blocks, [k, q] orientation, f32.

Device structure (per core) — single fused loop over the four 512-row
q-slices s, so projection (PE+DVE), softmax (ACT) and output projection (PE)
of neighbouring slices overlap instead of running as serial phases:

  for s in 0..3:
    DMA x-slice;  project q0..q3 (RoPE) — score chunks of the previous head
    interleaved between the d-matmuls so ACT exp runs concurrently;
    project k (RoPE), v (natural); then per head: diagonal score chunks,
    PV (probsT.T @ v_all, denominators from the ones column), normalize,
    PE-transpose into attnT — with deferred wo-blocks of slice s-1 spliced
    in wherever ACT needs catch-up time.
  drain the last slice's wo blocks.

Softmax skips max-subtraction: q,k rows are ~N(0,1) by construction, so
scores are ~N(0,1) after the folded 1/sqrt(hd) scale and exp() cannot
overflow in f32.
"""

import numpy as np

import concourse.bass as bass
import concourse.mybir as mybir
import concourse.tile as tile
from concourse import bacc
from concourse.masks import make_identity

F32 = mybir.dt.float32
BF16 = mybir.dt.bfloat16

B = 2
S = 2048
D = 2048
N_HEADS = 16
N_KV_HEADS = 4
HD = 128  # head dim
HC = N_HEADS // N_KV_HEADS  # q-heads per core (= per kv group) = 4
N_CORES = 8
NEG = -1e30

PB = 128       # partition block
SB = 512       # q-slice width / matmul free-dim slice
N_D = D // PB  # 16 contraction chunks over model dim
N_S = S // SB  # 4 q-slices
N_KB = S // PB # 16 k/q 128-blocks
DG = 4         # d-chunks per DMA group
N_G = N_D // DG
VBLK = 132     # v_all per-k-block column stride (128 v cols + ones + pad)
VN = 129       # PV matmul free dim (v cols + ones col)


def emit_core_kernel(nc, tc, io, repeat=1):
    """Emit one core's program. io: dict of dram tensor handles."""
    x2, wq2, wk2, wv2, wo2 = io["x2"], io["wq2"], io["wk2"], io["wv2"], io["wo2"]
    cs1, cs2, maskT, vpad, out = io["cs1"], io["cs2"], io["maskT"], io["vpad"], io["out"]

    with tc.tile_pool(name="consts", bufs=1) as consts:
        mask_sb = consts.tile([PB, PB], F32, tag="mask")
        nc.sync.dma_start(out=mask_sb[:, :], in_=maskT[:, :])
        ident = consts.tile([PB, PB], BF16, tag="ident")
        make_identity(nc, ident[:, :])

        for _rep in range(repeat):
            with (
                tc.tile_pool(name="wp", bufs=1) as wp,
                tc.tile_pool(name="qkv", bufs=1) as qkv,
                tc.tile_pool(name="xtp", bufs=1) as xtp,
                tc.tile_pool(name="wk_p", bufs=1) as wk_p,
                tc.tile_pool(name="ptp", bufs=1) as ptp,
                tc.tile_pool(name="accp", bufs=1, space="PSUM") as accp,
                tc.tile_pool(name="pssp", bufs=1, space="PSUM") as pssp,
                tc.tile_pool(name="psap", bufs=1, space="PSUM") as psap,
            ):
                cs1_sb = wp.tile([PB, S], BF16, tag="cs1")  # [cos; sin]
                cs2_sb = wp.tile([PB, S], BF16, tag="cs2")  # [sin; cos]
                wq_sb = wp.tile([PB, N_D * HC * HD], BF16, tag="wq")  # d-major
                wk_sb = wp.tile([PB, N_D * HD], BF16, tag="wk")
                wv_sb = wp.tile([PB, N_D * HD], BF16, tag="wv")
                wo_sb = wp.tile([PB, HC * D], BF16, tag="wo")  # h-major

                kT = qkv.tile([PB, S], BF16, tag="kT")
                v_all = qkv.tile([PB, N_KB * VBLK], BF16, tag="v")

                if _rep == 0:
                    # p-state warmup: run throwaway matmuls during the initial
                    # input-DMA stall so real work starts at full PE clock
                    warm = wk_p.tile([PB, SB], BF16, tag="warm", bufs=1)
                    nc.gpsimd.memset(warm[:, :], 0.0)
                    for _w in range(6):
                        pw = psap.tile([PB, SB], F32, tag="psa", bufs=2)
                        nc.tensor.matmul(
                            pw[:, :], ident[:, :], warm[:, :], start=True, stop=True
                        )

                def dma_wq(g, eng=None):
                    (eng or nc.sync).dma_start(
                        out=wq_sb[:, :].rearrange("p (d c) -> p d c", c=HC * HD)[
                            :, g * DG:(g + 1) * DG, :
                        ],
                        in_=wq2[:, g * DG:(g + 1) * DG, :],
                    )

                # ---------------- helpers ----------------
                def rope_evict(ps, s, dr, di):
                    # ps rows 0:64 = even half (re=a), 64:128 = odd half (im=b)
                    sl = slice(s * SB, (s + 1) * SB)
                    t1 = wk_p.tile([64, SB], F32, tag="t1", bufs=2)  # a*cos
                    t2 = wk_p.tile([64, SB], F32, tag="t2", bufs=2)  # b*sin
                    t3 = wk_p.tile([64, SB], F32, tag="t3", bufs=2)  # a*sin
                    t4 = wk_p.tile([64, SB], F32, tag="t4", bufs=2)  # b*cos
                    nc.vector.tensor_mul(t1[:, :], ps[0:64, :], cs1_sb[0:64, sl])
                    nc.vector.tensor_mul(t2[:, :], ps[64:128, :], cs1_sb[64:128, sl])
                    nc.vector.tensor_mul(t3[:, :], ps[0:64, :], cs2_sb[0:64, sl])
                    nc.vector.tensor_mul(t4[:, :], ps[64:128, :], cs2_sb[64:128, sl])
                    nc.gpsimd.tensor_sub(dr, t1[:, :], t2[:, :])
                    nc.gpsimd.tensor_add(di, t3[:, :], t4[:, :])

                pts = {}    # (h, j) -> probsT tile (current slice)
                pt_qa = {}  # (h, j) -> global q col of tile col 0
                cur = {}    # current slice's qT / attnT tiles

                def score_chunk(h, s, j):
                    def emit():
                        qa = max(j * PB, s * SB)
                        w = (s + 1) * SB - qa
                        pt = ptp.tile([PB, SB], BF16, tag=f"pt{h}_{j}", bufs=1)
                        pts[(h, j)] = pt
                        pt_qa[(h, j)] = qa
                        pss = pssp.tile([PB, SB], F32, tag="pss", bufs=2)
                        nc.tensor.matmul(
                            pss[:, 0:w],
                            kT[:, j * PB:(j + 1) * PB],
                            cur["qT"][:, h * SB + qa - s * SB: h * SB + qa - s * SB + w],
                            start=True,
                            stop=True,
                        )
                        if qa == j * PB:  # diagonal block in cols 0:PB
                            nc.vector.tensor_add(
                                pss[:, 0:PB], pss[:, 0:PB], mask_sb[:, :]
                            )
                        nc.scalar.activation(
                            pt[:, 0:w], pss[:, 0:w],
                            mybir.ActivationFunctionType.Exp,
                        )
                    return emit

                def emit_pv(h, i, s):
                    psa = psap.tile([PB, SB], F32, tag="psa", bufs=2)
                    for j in range(i + 1):
                        pt = pts[(h, j)]
                        off = i * PB - pt_qa[(h, j)]
                        nc.tensor.matmul(
                            psa[:, 0:VN],
                            pt[:, off:off + PB],
                            v_all[:, j * VBLK: j * VBLK + VN],
                            start=(j == 0),
                            stop=(j == i),
                        )
                    rinv = wk_p.tile([PB, 1], F32, tag="rinv", bufs=3)
                    nc.vector.reciprocal(rinv[:, :], psa[:, HD:HD + 1])
                    attn = wk_p.tile([PB, PB], BF16, tag="attn", bufs=3)
                    nc.vector.tensor_scalar_mul(attn[:, :], psa[:, 0:HD], rinv[:, :])
                    pst = pssp.tile([PB, SB], BF16, tag="pst", bufs=1)
                    nc.tensor.transpose(pst[:, 0:PB], attn[:, :], ident[:, :])
                    lo = h * SB + (i - N_S * s) * PB
                    cp = nc.scalar.copy if (i % 2 == 0) else nc.vector.tensor_copy
                    cp(cur["attnT"][:, lo:lo + PB], pst[:, 0:PB])

                def wo_chunk(aT, s, i, n0, ot, c, split):
                    def emit():
                        ps = accp.tile([PB, SB], F32, tag="acc", bufs=3)
                        for h in range(HC):
                            lo = h * SB + (i - N_S * s) * PB
                            nc.tensor.matmul(
                                ps[:, :],
                                aT[:, lo:lo + PB],
                                wo_sb[:, h * D + n0: h * D + n0 + SB],
                                start=(h == 0),
                                stop=(h == HC - 1),
                            )
                        cp = nc.scalar.copy if (c % 2 == 0) else nc.vector.tensor_copy
                        cp(ot[:, n0:n0 + SB], ps[:, :])
                        if split:
                            nc.gpsimd.dma_start(
                                out=out[i * PB:(i + 1) * PB, n0:n0 + SB],
                                in_=ot[:, n0:n0 + SB],
                            )
                        elif n0 + SB == D:
                            nc.gpsimd.dma_start(
                                out=out[i * PB:(i + 1) * PB, :], in_=ot[:, :]
                            )
                    return emit

                def make_wo_block(aT, s, i, split=False):
                    ot = wk_p.tile([PB, D], BF16, tag="ot", bufs=2)
                    return [
                        wo_chunk(aT, s, i, n0, ot, c, split)
                        for c, n0 in enumerate(range(0, D, SB))
                    ]

                # ---------------- main loop ----------------
                wo_queue = []      # deferred wo i-blocks (lists of 4 chunk thunks)
                score_fill = []    # FIFO of (h, thunk): emitted score chunks are
                                   # spaced out so ACT exp keeps pace with PE

                def pull(n=1):
                    for _ in range(n):
                        if score_fill:
                            score_fill.pop(0)[1]()

                def drain_upto(h):
                    while score_fill and score_fill[0][0] <= h:
                        score_fill.pop(0)[1]()

                for s in range(N_S):
                    qT_s = qkv.tile([PB, HC * SB], BF16, tag="qT", bufs=2)
                    attnT_s = qkv.tile([PB, HC * SB], BF16, tag="attnT", bufs=2)
                    cur["qT"], cur["attnT"] = qT_s, attnT_s
                    xts = []
                    if s == 0:
                        # split the first x/wq group into half-size DMAs on two
                        # queues: the first q-matmul starts after ~2 small
                        # transfers instead of two full 512KB groups
                        xtg = xtp.tile([PB, DG * SB], BF16, tag="xt0", bufs=2)
                        xts.append(xtg)
                        wq3 = wq_sb[:, :].rearrange("p (d c) -> p d c", c=HC * HD)
                        for half in range(2):
                            dsl = slice(2 * half, 2 * half + 2)
                            csl = slice(2 * half * SB, (2 * half + 2) * SB)
                            nc.sync.dma_start(
                                out=xtg[:, csl].rearrange("p (d c) -> p d c", c=SB),
                                in_=x2[:, dsl, 0:SB],
                            )
                            nc.scalar.dma_start(
                                out=wq3[:, dsl, :], in_=wq2[:, dsl, :]
                            )
                        nc.scalar.dma_start(out=cs1_sb[:, :], in_=cs1[:, :])
                        nc.scalar.dma_start(out=cs2_sb[:, :], in_=cs2[:, :])
                    for g in range(0 if s else 1, N_G):
                        xtg = xtp.tile([PB, DG * SB], BF16, tag=f"xt{g}", bufs=2)
                        xq_eng = nc.sync if g < 2 else nc.scalar
                        xq_eng.dma_start(
                            out=xtg[:, :].rearrange("p (d c) -> p d c", c=SB),
                            in_=x2[:, g * DG:(g + 1) * DG, s * SB:(s + 1) * SB],
                        )
                        xts.append(xtg)
                        if s == 0:
                            dma_wq(g)
                    if s == 0:
                        nc.gpsimd.dma_start(
                            out=wk_sb[:, :].rearrange("p (d c) -> p d c", c=HD),
                            in_=wk2[:, :, :],
                        )
                        nc.gpsimd.dma_start(
                            out=wv_sb[:, :].rearrange("p (d c) -> p d c", c=HD),
                            in_=wv2[:, :, :],
                        )
                        nc.scalar.dma_start(
                            out=v_all[:, :].rearrange("p (j c) -> p j c", c=VBLK)[
                                :, :, HD:VBLK
                            ],
                            in_=vpad[:, :, :],
                        )
                        for gh in range(2):
                            nc.gpsimd.dma_start(
                                out=wo_sb[:, :].rearrange("p (h c) -> p h c", c=D)[
                                    :, gh * 2:(gh + 1) * 2, :
                                ],
                                in_=wo2[:, gh * 2:(gh + 1) * 2, :],
                            )

                    def xslice(d, lo=0, w=SB):
                        g, t = divmod(d, DG)
                        return xts[g][:, t * SB + lo: t * SB + lo + w]

                    # -- projections; pending score chunks spaced in every 3rd
                    # d-matmul so ACT exp keeps pace without backpressuring PE --
                    def q_matmul(ps, h, d):
                        nc.tensor.matmul(
                            ps[:, :],
                            wq_sb[:, d * HC * HD + h * HD: d * HC * HD + (h + 1) * HD],
                            xslice(d),
                            start=(d == 0),
                            stop=(d == N_D - 1),
                        )

                    def q_evict(ps, h):
                        rope_evict(
                            ps, s,
                            cur["qT"][0:64, h * SB:(h + 1) * SB],
                            cur["qT"][64:128, h * SB:(h + 1) * SB],
                        )
                        for j in range(N_S * s):
                            score_fill.append((h, score_chunk(h, s, j)))

                    def proj_q(h, pool=None):
                        if pool is None:
                            ps = accp.tile([PB, SB], F32, tag="acc", bufs=3)
                        else:
                            ps = psap.tile([PB, SB], F32, tag="psa", bufs=2)
                        for d in range(N_D):
                            q_matmul(ps, h, d)
                            if d % 3 == 2:
                                pull(1)
                        q_evict(ps, h)

                    if s == 0:
                        # two-pass g-outer start: q0/q1/q2 accumulate per DMA
                        # group so the PE consumes x/wq chunks as they land
                        ps0 = accp.tile([PB, SB], F32, tag="acc", bufs=3)
                        ps1 = accp.tile([PB, SB], F32, tag="acc", bufs=3)
                        ps2 = accp.tile([PB, SB], F32, tag="acc", bufs=3)
                        for g in range(N_G):
                            for t, psq in enumerate((ps0, ps1, ps2)):
                                for dd in range(DG):
                                    q_matmul(psq, t, g * DG + dd)
                                if g == N_G - 1:
                                    q_evict(psq, t)  # eager: frees the psum ring
                        proj_q(3, pool="psa")  # psa ring is idle until PV
                    else:
                        for h in range(HC):
                            proj_q(h)
                    # k projection
                    ps = accp.tile([PB, SB], F32, tag="acc", bufs=3)
                    for d in range(N_D):
                        nc.tensor.matmul(
                            ps[:, :],
                            wk_sb[:, d * HD:(d + 1) * HD],
                            xslice(d),
                            start=(d == 0),
                            stop=(d == N_D - 1),
                        )
                        if d % 3 == 2:
                            pull(1)
                    rope_evict(
                        ps, s,
                        kT[0:64, s * SB:(s + 1) * SB],
                        kT[64:128, s * SB:(s + 1) * SB],
                    )
                    # v in natural [seq, hd] orientation, one 128-block per j
                    for t in range(SB // PB):
                        j = N_S * s + t
                        if s == 0 and t < 2:
                            # scores haven't started yet at s=0: borrow the
                            # idle pss ring so v doesn't wait on eviction
                            # drains to free the acc ring
                            psv = pssp.tile([PB, SB], F32, tag="pss", bufs=2)
                        else:
                            psv = accp.tile([PB, SB], F32, tag="acc", bufs=3)
                        for d in range(N_D):
                            nc.tensor.matmul(
                                psv[:, 0:HD],
                                xslice(d, t * PB, PB),
                                wv_sb[:, d * HD:(d + 1) * HD],
                                start=(d == 0),
                                stop=(d == N_D - 1),
                            )
                            if (t * N_D + d) % 12 == 11:
                                pull(1)
                        nc.scalar.copy(v_all[:, j * VBLK: j * VBLK + HD], psv[:, 0:HD])

                    # -- attention: per head, wo blocks of slice s-1 spliced in --
                    for h in range(HC):
                        if wo_queue:
                            for t_ in wo_queue.pop(0):
                                t_()
                                pull(1)
                        drain_upto(h)
                        for j in range(N_S * s, N_S * s + N_S):
                            score_chunk(h, s, j)()
                        for i in range(N_S * s, N_S * s + N_S):
                            emit_pv(h, i, s)
                            pull(1)
                    for i in range(N_S * s, N_S * s + N_S):
                        wo_queue.append(
                            make_wo_block(cur["attnT"], s, i, split=(s == N_S - 1))
                        )
                # drain the last slice's output blocks
                while wo_queue:
                    for t_ in wo_queue.pop(0):
                        t_()


def build_nc(repeat=1):
    nc = bacc.Bacc("TRN2", target_bir_lowering=False, debug=False, num_devices=N_CORES)
    io = {
        "x2": nc.dram_tensor("x2", [PB, N_D, S], BF16, kind="ExternalInput"),
        "wq2": nc.dram_tensor("wq2", [PB, N_D, HC * HD], BF16, kind="ExternalInput"),
        "wk2": nc.dram_tensor("wk2", [PB, N_D, HD], BF16, kind="ExternalInput"),
        "wv2": nc.dram_tensor("wv2", [PB, N_D, HD], BF16, kind="ExternalInput"),
        "wo2": nc.dram_tensor("wo2", [PB, HC, D], BF16, kind="ExternalInput"),
        "cs1": nc.dram_tensor("cs1", [PB, S], BF16, kind="ExternalInput"),
        "cs2": nc.dram_tensor("cs2", [PB, S], BF16, kind="ExternalInput"),
        "maskT": nc.dram_tensor("maskT", [PB, PB], F32, kind="ExternalInput"),
        "vpad": nc.dram_tensor("vpad", [PB, N_KB, VBLK - HD], BF16, kind="ExternalInput"),
        "out": nc.dram_tensor("out", [S, D], BF16, kind="ExternalOutput"),
    }
    with tile.TileContext(nc) as tc:
        emit_core_kernel(nc, tc, io, repeat=repeat)
    nc.compile()
    return nc


# ---------------------------------------------------------------------------
# host-side sharding + execution
# ---------------------------------------------------------------------------

_HALFSPLIT = np.concatenate([np.arange(0, HD, 2), np.arange(1, HD, 2)])


def _bf16():
    import ml_dtypes
    return ml_dtypes.bfloat16


def _tile_p(a, cols):
    """[D, cols] -> [128, N_D, cols] with [p, d, :] = a[d*128+p, :]."""
    return np.ascontiguousarray(
        np.asarray(a, np.float32).reshape(-1, PB, cols).transpose(1, 0, 2)
    )


def make_core_inputs(x, wq, wk, wv, wo, freqs_cos, freqs_sin):
    """Build the 8 per-core input dicts (numpy, host-side)."""
    BF = _bf16()
    scale = np.float32(1.0 / np.sqrt(HD))
    maskT = np.where(
        np.arange(PB)[None, :] >= np.arange(PB)[:, None], np.float32(0), np.float32(NEG)
    ).astype(np.float32)  # [k, q]: masked where q < k
    vpad = np.zeros((PB, N_KB, VBLK - HD), BF)
    vpad[:, :, 0] = 1

    x2s, cs1s, cs2s = [], [], []
    for b in range(B):
        xb = np.asarray(x[b], np.float32)  # [S, D]
        x2s.append(_tile_p(xb.T, S).astype(BF))
        cosb = np.asarray(freqs_cos[b], np.float32).T  # [64, S]
        sinb = np.asarray(freqs_sin[b], np.float32).T
        cs1s.append(np.concatenate([cosb, sinb], axis=0).astype(BF))
        cs2s.append(np.concatenate([sinb, cosb], axis=0).astype(BF))

    in_maps = []
    for c in range(N_CORES):
        b, g = divmod(c, N_KV_HEADS)
        qcols = np.concatenate([(HC * g + h) * HD + _HALFSPLIT for h in range(HC)])
        wq_c = np.ascontiguousarray(np.asarray(wq, np.float32)[:, qcols]) * scale
        wk_c = np.ascontiguousarray(np.asarray(wk, np.float32)[:, g * HD + _HALFSPLIT])
        wv_c = np.ascontiguousarray(np.asarray(wv, np.float32)[:, g * HD:(g + 1) * HD])
        wo_c = np.ascontiguousarray(
            np.asarray(wo, np.float32)[g * HC * HD:(g + 1) * HC * HD, :]
        )
        in_maps.append(
            {
                "x2": x2s[b],
                "wq2": _tile_p(wq_c, HC * HD).astype(BF),
                "wk2": _tile_p(wk_c, HD).astype(BF),
                "wv2": _tile_p(wv_c, HD).astype(BF),
                "wo2": _tile_p(wo_c, D).astype(BF),
                "cs1": cs1s[b],
                "cs2": cs2s[b],
                "maskT": maskT,
                "vpad": vpad,
            }
        )
    return in_maps


_CACHE = {}


def get_runner(repeat=1, chain=1):
    """Build (once) the Bass module and a cached jitted 8-core executor."""
    if (repeat, chain) in _CACHE:
        return _CACHE[(repeat, chain)]
    import jax
    from jax.sharding import Mesh, PartitionSpec
    from jax.experimental.shard_map import shard_map
    from concourse.bass2jax import (
        _bass_exec_p,
        install_neuronx_cc_hook,
        partition_id_tensor,
    )

    nc = build_nc(repeat=repeat)
    install_neuronx_cc_hook()
    partition_name = nc.partition_id_tensor.name if nc.partition_id_tensor else None
    in_names, out_names, out_avals = [], [], []
    for alloc in nc.m.functions[0].allocations:
        if not isinstance(alloc, mybir.MemoryLocationSet):
            continue
        name = alloc.memorylocations[0].name
        if alloc.kind == "ExternalInput":
            if name != partition_name:
                in_names.append(name)
        elif alloc.kind == "ExternalOutput":
            out_names.append(name)
            out_avals.append(
                jax.core.ShapedArray(tuple(alloc.tensor_shape), mybir.dt.np(alloc.dtype))
            )
    n_params = len(in_names)
    n_outs = len(out_avals)
    all_in_names = list(in_names) + list(out_names)
    if partition_name is not None:
        all_in_names.append(partition_name)

    def _body(*args):
        operands = list(args)
        if partition_name is not None:
            operands.append(partition_id_tensor())
        outs = _bass_exec_p.bind(
            *operands,
            out_avals=tuple(out_avals),
            in_names=tuple(all_in_names),
            out_names=tuple(out_names),
            lowering_input_output_aliases=(),
            sim_require_finite=True,
            sim_require_nnan=True,
            nc=nc,
        )
        return tuple(outs)

    devices = jax.devices()[:N_CORES]
    mesh = Mesh(np.asarray(devices), ("core",))
    in_specs = (PartitionSpec("core"),) * (n_params + n_outs)
    out_specs = (PartitionSpec("core"),) * n_outs

    def _chain(*args):
        ins, outs = args[:n_params], args[n_params:]
        for _ in range(chain):
            outs = _body(*ins, *outs)
        return outs

    fn = jax.jit(
        shard_map(_chain, mesh=mesh, in_specs=in_specs, out_specs=out_specs, check_rep=False),
        keep_unused=True,
    )

    from jax.sharding import NamedSharding

    sh = NamedSharding(mesh, PartitionSpec("core"))

    def prepare(in_maps):
        concat_in = [
            np.concatenate([m[name] for m in in_maps], axis=0) for name in in_names
        ]
        concat_zeros = [
            np.zeros((N_CORES * a.shape[0], *a.shape[1:]), a.dtype) for a in out_avals
        ]
        return [jax.device_put(a, sh) for a in concat_in + concat_zeros]

    def run_dev(dev_args):
        out_arrs = fn(*dev_args)
        jax.block_until_ready(out_arrs)
        return out_arrs

    def run(in_maps):
        out_arrs = run_dev(prepare(in_maps))
        return np.asarray(out_arrs[0]).reshape(N_CORES, S, D)

    run.prepare = prepare
    run.run_dev = run_dev
    run.fn = fn
    _CACHE[(repeat, chain)] = run
    return run


def kernel(x, wq, wk, wv, wo, freqs_cos, freqs_sin):
    in_maps = make_core_inputs(x, wq, wk, wv, wo, freqs_cos, freqs_sin)
    run = get_runner(repeat=1)
    partials = run(in_maps).astype(np.float32)  # [8, S, D]
    out = np.stack(
        [partials[b * N_KV_HEADS:(b + 1) * N_KV_HEADS].sum(axis=0) for b in range(B)]
    )
    return out.astype(np.float32)


# revision 48
# speedup vs baseline: 1.0889x; 1.0889x over previous
"""GQA attention (B=2, S=2048, D=2048, 16 q-heads / 4 kv-heads, RoPE, causal)
for 8 Trainium2 NeuronCores.

Sharding: core c = 4*b + g handles batch b and GQA group g (q-heads 4g..4g+3,
kv-head g). Each core computes q/k/v projections for its group, RoPE, causal
attention, and the partial output projection attn @ wo[rows of its heads].
The host sums the 4 partials per batch (the only cross-core reduction).

All matmul operands are bf16 (PE runs 1 cycle/row at any free size, DMA bytes
halve); PSUM accumulation stays f32.  Output is written bf16 and upcast on the
host.  rel-err budget 2e-2; measured ~5e-3.

Host-side preprocessing folded into the inputs:
- x / weights pre-tiled to [128 partitions, d-chunk, cols] so each DMA group
  lands in SBUF layout directly (p-first iteration on both sides).
- wq/wk columns permuted per head from interleaved (even,odd) RoPE pairs to
  half-split ([evens | odds]); 1/sqrt(head_dim) folded into wq.
- cs1/cs2: [cos;sin] and [sin;cos] row stacks, so the 4 RoPE products read
  the psum halves against partition-aligned cos/sin rows (the BIR verifier
  requires equal base partitions only when BOTH inputs are SBUF; psum inputs
  are exempt); the two combines are SBUF-aligned and run on GpSimd, which is
  otherwise idle.
- v is projected in NATURAL [seq, hd] orientation (lhsT = x-chunk, rhs = wv
  chunk) so no PE transposes are needed for the PV rhs.
- vpad: 4 tail cols per 132-col v block; col 128 is an all-ones column so the
  PV matmul emits softmax denominators for free (psa col 128 = row sums).
- Causal mask for diagonal 128x128 blocks, [k, q] orientation, f32.

Device structure (per core) — single fused loop over the four 512-row
q-slices s, so projection (PE+DVE), softmax (ACT) and output projection (PE)
of neighbouring slices overlap instead of running as serial phases:

  for s in 0..3:
    DMA x-slice;  project q0..q3 (RoPE) — score chunks of the previous head
    interleaved between the d-matmuls so ACT exp runs concurrently;
    project k (RoPE), v (natural); then per head: diagonal score chunks,
    PV (probsT.T @ v_all, denominators from the ones column), normalize,
    PE-transpose into attnT — with deferred wo-blocks of slice s-1 spliced
    in wherever ACT needs catch-up time.
  drain the last slice's wo blocks.

Softmax skips max-subtraction: q,k rows are ~N(0,1) by construction, so
scores are ~N(0,1) after the folded 1/sqrt(hd) scale and exp() cannot
overflow in f32.
"""

import numpy as np

import concourse.bass as bass
import concourse.mybir as mybir
import concourse.tile as tile
from concourse import bacc
from concourse.masks import make_identity

F32 = mybir.dt.float32
BF16 = mybir.dt.bfloat16

B = 2
S = 2048
D = 2048
N_HEADS = 16
N_KV_HEADS = 4
HD = 128  # head dim
HC = N_HEADS // N_KV_HEADS  # q-heads per core (= per kv group) = 4
N_CORES = 8
NEG = -1e30

PB = 128       # partition block
SB = 512       # q-slice width / matmul free-dim slice
N_D = D // PB  # 16 contraction chunks over model dim
N_S = S // SB  # 4 q-slices
N_KB = S // PB # 16 k/q 128-blocks
DG = 4         # d-chunks per DMA group
N_G = N_D // DG
VBLK = 132     # v_all per-k-block column stride (128 v cols + ones + pad)
VN = 129       # PV matmul free dim (v cols + ones col)


def emit_core_kernel(nc, tc, io, repeat=1):
    """Emit one core's program. io: dict of dram tensor handles."""
    x2, wq2, wk2, wv2, wo2 = io["x2"], io["wq2"], io["wk2"], io["wv2"], io["wo2"]
    cs1, cs2, maskT, vpad, out = io["cs1"], io["cs2"], io["maskT"], io["vpad"], io["out"]

    with tc.tile_pool(name="consts", bufs=1) as consts:
        mask_sb = consts.tile([PB, PB], F32, tag="mask")
        nc.sync.dma_start(out=mask_sb[:, :], in_=maskT[:, :])
        ident = consts.tile([PB, PB], BF16, tag="ident")
        make_identity(nc, ident[:, :])

        for _rep in range(repeat):
            with (
                tc.tile_pool(name="wp", bufs=1) as wp,
                tc.tile_pool(name="qkv", bufs=1) as qkv,
                tc.tile_pool(name="xtp", bufs=1) as xtp,
                tc.tile_pool(name="wk_p", bufs=1) as wk_p,
                tc.tile_pool(name="ptp", bufs=1) as ptp,
                tc.tile_pool(name="accp", bufs=1, space="PSUM") as accp,
                tc.tile_pool(name="pssp", bufs=1, space="PSUM") as pssp,
                tc.tile_pool(name="psap", bufs=1, space="PSUM") as psap,
            ):
                cs1_sb = wp.tile([PB, S], BF16, tag="cs1")  # [cos; sin]
                cs2_sb = wp.tile([PB, S], BF16, tag="cs2")  # [sin; cos]
                wq_sb = wp.tile([PB, N_D * HC * HD], BF16, tag="wq")  # d-major
                wk_sb = wp.tile([PB, N_D * HD], BF16, tag="wk")
                wv_sb = wp.tile([PB, N_D * HD], BF16, tag="wv")
                wo_sb = wp.tile([PB, HC * D], BF16, tag="wo")  # h-major

                kT = qkv.tile([PB, S], BF16, tag="kT")
                v_all = qkv.tile([PB, N_KB * VBLK], BF16, tag="v")

                if _rep == 0:
                    # p-state warmup: run throwaway matmuls during the initial
                    # input-DMA stall so real work starts at full PE clock
                    warm = wk_p.tile([PB, SB], BF16, tag="warm", bufs=1)
                    nc.gpsimd.memset(warm[:, :], 0.0)
                    for _w in range(6):
                        pw = psap.tile([PB, SB], F32, tag="psa", bufs=2)
                        nc.tensor.matmul(
                            pw[:, :], ident[:, :], warm[:, :], start=True, stop=True
                        )

                def dma_wq(g, eng=None):
                    (eng or nc.sync).dma_start(
                        out=wq_sb[:, :].rearrange("p (d c) -> p d c", c=HC * HD)[
                            :, g * DG:(g + 1) * DG, :
                        ],
                        in_=wq2[:, g * DG:(g + 1) * DG, :],
                    )

                # ---------------- helpers ----------------
                def rope_evict(ps, s, dr, di):
                    # ps rows 0:64 = even half (re=a), 64:128 = odd half (im=b)
                    sl = slice(s * SB, (s + 1) * SB)
                    t1 = wk_p.tile([64, SB], F32, tag="t1", bufs=2)  # a*cos
                    t2 = wk_p.tile([64, SB], F32, tag="t2", bufs=2)  # b*sin
                    t3 = wk_p.tile([64, SB], F32, tag="t3", bufs=2)  # a*sin
                    t4 = wk_p.tile([64, SB], F32, tag="t4", bufs=2)  # b*cos
                    nc.vector.tensor_mul(t1[:, :], ps[0:64, :], cs1_sb[0:64, sl])
                    nc.vector.tensor_mul(t2[:, :], ps[64:128, :], cs1_sb[64:128, sl])
                    nc.vector.tensor_mul(t3[:, :], ps[0:64, :], cs2_sb[0:64, sl])
                    nc.vector.tensor_mul(t4[:, :], ps[64:128, :], cs2_sb[64:128, sl])
                    nc.gpsimd.tensor_sub(dr, t1[:, :], t2[:, :])
                    nc.gpsimd.tensor_add(di, t3[:, :], t4[:, :])

                pts = {}    # (h, j) -> probsT tile (current slice)
                pt_qa = {}  # (h, j) -> global q col of tile col 0
                cur = {}    # current slice's qT / attnT tiles
                pend_t = [] # deferred attn -> attnT crossbar transposes

                def score_chunk(h, s, j):
                    def emit():
                        qa = max(j * PB, s * SB)
                        w = (s + 1) * SB - qa
                        pt = ptp.tile([PB, SB], BF16, tag=f"pt{h}_{j}", bufs=1)
                        pts[(h, j)] = pt
                        pt_qa[(h, j)] = qa
                        pss = pssp.tile([PB, SB], F32, tag="pss", bufs=2)
                        nc.tensor.matmul(
                            pss[:, 0:w],
                            kT[:, j * PB:(j + 1) * PB],
                            cur["qT"][:, h * SB + qa - s * SB: h * SB + qa - s * SB + w],
                            start=True,
                            stop=True,
                        )
                        if qa == j * PB:  # diagonal block in cols 0:PB
                            nc.vector.tensor_add(
                                pss[:, 0:PB], pss[:, 0:PB], mask_sb[:, :]
                            )
                        nc.scalar.activation(
                            pt[:, 0:w], pss[:, 0:w],
                            mybir.ActivationFunctionType.Exp,
                        )
                    return emit

                def emit_pv(h, i, s):
                    psa = psap.tile([PB, SB], F32, tag="psa", bufs=2)
                    for j in range(i + 1):
                        pt = pts[(h, j)]
                        off = i * PB - pt_qa[(h, j)]
                        nc.tensor.matmul(
                            psa[:, 0:VN],
                            pt[:, off:off + PB],
                            v_all[:, j * VBLK: j * VBLK + VN],
                            start=(j == 0),
                            stop=(j == i),
                        )
                    rinv = wk_p.tile([PB, 1], F32, tag="rinv", bufs=3)
                    nc.vector.reciprocal(rinv[:, :], psa[:, HD:HD + 1])
                    attn = wk_p.tile([PB, PB], BF16, tag="attn", bufs=20)
                    nc.vector.tensor_scalar_mul(attn[:, :], psa[:, 0:HD], rinv[:, :])
                    lo = h * SB + (i - N_S * s) * PB
                    if s < N_S - 1:
                        # defer: DMA crossbar transpose emitted at the top of
                        # slice s+1 (after its x prefetch) so neither the PE
                        # nor the prefetch queue pays for it here
                        pend_t.append((cur["attnT"], lo, attn))
                    else:
                        # last slice: inline PE transpose keeps the tail drain
                        # unblocked
                        pst = pssp.tile([PB, SB], BF16, tag="pst", bufs=1)
                        nc.tensor.transpose(pst[:, 0:PB], attn[:, :], ident[:, :])
                        cp = nc.scalar.copy if (i % 2 == 0) else nc.vector.tensor_copy
                        cp(cur["attnT"][:, lo:lo + PB], pst[:, 0:PB])

                def wo_chunk(aT, s, i, n0, ot, c, split):
                    def emit():
                        ps = accp.tile([PB, SB], F32, tag="acc", bufs=3)
                        for h in range(HC):
                            lo = h * SB + (i - N_S * s) * PB
                            nc.tensor.matmul(
                                ps[:, :],
                                aT[:, lo:lo + PB],
                                wo_sb[:, h * D + n0: h * D + n0 + SB],
                                start=(h == 0),
                                stop=(h == HC - 1),
                            )
                        cp = nc.scalar.copy if (c % 2 == 0) else nc.vector.tensor_copy
                        cp(ot[:, n0:n0 + SB], ps[:, :])
                        if split:
                            nc.gpsimd.dma_start(
                                out=out[i * PB:(i + 1) * PB, n0:n0 + SB],
                                in_=ot[:, n0:n0 + SB],
                            )
                        elif n0 + SB == D:
                            nc.gpsimd.dma_start(
                                out=out[i * PB:(i + 1) * PB, :], in_=ot[:, :]
                            )
                    return emit

                def make_wo_block(aT, s, i, split=False):
                    ot = wk_p.tile([PB, D], BF16, tag="ot", bufs=2)
                    return [
                        wo_chunk(aT, s, i, n0, ot, c, split)
                        for c, n0 in enumerate(range(0, D, SB))
                    ]

                # ---------------- main loop ----------------
                wo_queue = []      # deferred wo i-blocks (lists of 4 chunk thunks)
                score_fill = []    # FIFO of (h, thunk): emitted score chunks are
                                   # spaced out so ACT exp keeps pace with PE

                def pull(n=1):
                    for _ in range(n):
                        if score_fill:
                            score_fill.pop(0)[1]()

                def drain_upto(h):
                    while score_fill and score_fill[0][0] <= h:
                        score_fill.pop(0)[1]()

                for s in range(N_S):
                    qT_s = qkv.tile([PB, HC * SB], BF16, tag="qT", bufs=2)
                    attnT_s = qkv.tile([PB, HC * SB], BF16, tag="attnT", bufs=2)
                    cur["qT"], cur["attnT"] = qT_s, attnT_s
                    xts = []
                    if s == 0:
                        # split the first x/wq group into half-size DMAs on two
                        # queues: the first q-matmul starts after ~2 small
                        # transfers instead of two full 512KB groups
                        xtg = xtp.tile([PB, DG * SB], BF16, tag="xt0", bufs=2)
                        xts.append(xtg)
                        wq3 = wq_sb[:, :].rearrange("p (d c) -> p d c", c=HC * HD)
                        for half in range(2):
                            dsl = slice(2 * half, 2 * half + 2)
                            csl = slice(2 * half * SB, (2 * half + 2) * SB)
                            nc.sync.dma_start(
                                out=xtg[:, csl].rearrange("p (d c) -> p d c", c=SB),
                                in_=x2[:, dsl, 0:SB],
                            )
                            nc.scalar.dma_start(
                                out=wq3[:, dsl, :], in_=wq2[:, dsl, :]
                            )
                        nc.scalar.dma_start(out=cs1_sb[:, :], in_=cs1[:, :])
                        nc.scalar.dma_start(out=cs2_sb[:, :], in_=cs2[:, :])
                    for g in range(0 if s else 1, N_G):
                        xtg = xtp.tile([PB, DG * SB], BF16, tag=f"xt{g}", bufs=2)
                        xq_eng = nc.sync if g < 2 else nc.scalar
                        xq_eng.dma_start(
                            out=xtg[:, :].rearrange("p (d c) -> p d c", c=SB),
                            in_=x2[:, g * DG:(g + 1) * DG, s * SB:(s + 1) * SB],
                        )
                        xts.append(xtg)
                        if s == 0:
                            dma_wq(g)
                    if s == 0:
                        nc.gpsimd.dma_start(
                            out=wk_sb[:, :].rearrange("p (d c) -> p d c", c=HD),
                            in_=wk2[:, :, :],
                        )
                        nc.gpsimd.dma_start(
                            out=wv_sb[:, :].rearrange("p (d c) -> p d c", c=HD),
                            in_=wv2[:, :, :],
                        )
                        nc.scalar.dma_start(
                            out=v_all[:, :].rearrange("p (j c) -> p j c", c=VBLK)[
                                :, :, HD:VBLK
                            ],
                            in_=vpad[:, :, :],
                        )
                        for gh in range(2):
                            nc.gpsimd.dma_start(
                                out=wo_sb[:, :].rearrange("p (h c) -> p h c", c=D)[
                                    :, gh * 2:(gh + 1) * 2, :
                                ],
                                in_=wo2[:, gh * 2:(gh + 1) * 2, :],
                            )

                    for aTt, lo_, attn_ in pend_t:
                        nc.sync.dma_start_transpose(aTt[:, lo_:lo_ + PB], attn_[:, :])
                    pend_t.clear()

                    def xslice(d, lo=0, w=SB):
                        g, t = divmod(d, DG)
                        return xts[g][:, t * SB + lo: t * SB + lo + w]

                    # -- projections; pending score chunks spaced in every 3rd
                    # d-matmul so ACT exp keeps pace without backpressuring PE --
                    def q_matmul(ps, h, d):
                        nc.tensor.matmul(
                            ps[:, :],
                            wq_sb[:, d * HC * HD + h * HD: d * HC * HD + (h + 1) * HD],
                            xslice(d),
                            start=(d == 0),
                            stop=(d == N_D - 1),
                        )

                    def q_evict(ps, h):
                        rope_evict(
                            ps, s,
                            cur["qT"][0:64, h * SB:(h + 1) * SB],
                            cur["qT"][64:128, h * SB:(h + 1) * SB],
                        )
                        for j in range(N_S * s):
                            score_fill.append((h, score_chunk(h, s, j)))

                    def proj_q(h, pool=None):
                        if pool is None:
                            ps = accp.tile([PB, SB], F32, tag="acc", bufs=3)
                        else:
                            ps = psap.tile([PB, SB], F32, tag="psa", bufs=2)
                        for d in range(N_D):
                            q_matmul(ps, h, d)
                            if d % 3 == 2:
                                pull(1)
                        q_evict(ps, h)

                    if s == 0:
                        # two-pass g-outer start: q0/q1/q2 accumulate per DMA
                        # group so the PE consumes x/wq chunks as they land
                        ps0 = accp.tile([PB, SB], F32, tag="acc", bufs=3)
                        ps1 = accp.tile([PB, SB], F32, tag="acc", bufs=3)
                        ps2 = accp.tile([PB, SB], F32, tag="acc", bufs=3)
                        for g in range(N_G):
                            for t, psq in enumerate((ps0, ps1, ps2)):
                                for dd in range(DG):
                                    q_matmul(psq, t, g * DG + dd)
                                if g == N_G - 1:
                                    q_evict(psq, t)  # eager: frees the psum ring
                        proj_q(3, pool="psa")  # psa ring is idle until PV
                    else:
                        for h in range(HC):
                            proj_q(h)
                    # k projection
                    ps = accp.tile([PB, SB], F32, tag="acc", bufs=3)
                    for d in range(N_D):
                        nc.tensor.matmul(
                            ps[:, :],
                            wk_sb[:, d * HD:(d + 1) * HD],
                            xslice(d),
                            start=(d == 0),
                            stop=(d == N_D - 1),
                        )
                        if d % 3 == 2:
                            pull(1)
                    rope_evict(
                        ps, s,
                        kT[0:64, s * SB:(s + 1) * SB],
                        kT[64:128, s * SB:(s + 1) * SB],
                    )
                    # v in natural [seq, hd] orientation, one 128-block per j
                    for t in range(SB // PB):
                        j = N_S * s + t
                        if s == 0 and t < 2:
                            # scores haven't started yet at s=0: borrow the
                            # idle pss ring so v doesn't wait on eviction
                            # drains to free the acc ring
                            psv = pssp.tile([PB, SB], F32, tag="pss", bufs=2)
                        else:
                            psv = accp.tile([PB, SB], F32, tag="acc", bufs=3)
                        for d in range(N_D):
                            nc.tensor.matmul(
                                psv[:, 0:HD],
                                xslice(d, t * PB, PB),
                                wv_sb[:, d * HD:(d + 1) * HD],
                                start=(d == 0),
                                stop=(d == N_D - 1),
                            )
                            if (t * N_D + d) % 12 == 11:
                                pull(1)
                        nc.scalar.copy(v_all[:, j * VBLK: j * VBLK + HD], psv[:, 0:HD])

                    # -- attention: per head, wo blocks of slice s-1 spliced in --
                    for h in range(HC):
                        if wo_queue:
                            for t_ in wo_queue.pop(0):
                                t_()
                                pull(1)
                        drain_upto(h)
                        for j in range(N_S * s, N_S * s + N_S):
                            score_chunk(h, s, j)()
                        for i in range(N_S * s, N_S * s + N_S):
                            emit_pv(h, i, s)
                            pull(1)
                    for i in range(N_S * s, N_S * s + N_S):
                        wo_queue.append(
                            make_wo_block(cur["attnT"], s, i, split=(s == N_S - 1))
                        )
                # drain the last slice's output blocks
                while wo_queue:
                    for t_ in wo_queue.pop(0):
                        t_()


def build_nc(repeat=1):
    nc = bacc.Bacc("TRN2", target_bir_lowering=False, debug=False, num_devices=N_CORES)
    io = {
        "x2": nc.dram_tensor("x2", [PB, N_D, S], BF16, kind="ExternalInput"),
        "wq2": nc.dram_tensor("wq2", [PB, N_D, HC * HD], BF16, kind="ExternalInput"),
        "wk2": nc.dram_tensor("wk2", [PB, N_D, HD], BF16, kind="ExternalInput"),
        "wv2": nc.dram_tensor("wv2", [PB, N_D, HD], BF16, kind="ExternalInput"),
        "wo2": nc.dram_tensor("wo2", [PB, HC, D], BF16, kind="ExternalInput"),
        "cs1": nc.dram_tensor("cs1", [PB, S], BF16, kind="ExternalInput"),
        "cs2": nc.dram_tensor("cs2", [PB, S], BF16, kind="ExternalInput"),
        "maskT": nc.dram_tensor("maskT", [PB, PB], F32, kind="ExternalInput"),
        "vpad": nc.dram_tensor("vpad", [PB, N_KB, VBLK - HD], BF16, kind="ExternalInput"),
        "out": nc.dram_tensor("out", [S, D], BF16, kind="ExternalOutput"),
    }
    with tile.TileContext(nc) as tc:
        emit_core_kernel(nc, tc, io, repeat=repeat)
    nc.compile()
    return nc


# ---------------------------------------------------------------------------
# host-side sharding + execution
# ---------------------------------------------------------------------------

_HALFSPLIT = np.concatenate([np.arange(0, HD, 2), np.arange(1, HD, 2)])


def _bf16():
    import ml_dtypes
    return ml_dtypes.bfloat16


def _tile_p(a, cols):
    """[D, cols] -> [128, N_D, cols] with [p, d, :] = a[d*128+p, :]."""
    return np.ascontiguousarray(
        np.asarray(a, np.float32).reshape(-1, PB, cols).transpose(1, 0, 2)
    )


def make_core_inputs(x, wq, wk, wv, wo, freqs_cos, freqs_sin):
    """Build the 8 per-core input dicts (numpy, host-side)."""
    BF = _bf16()
    scale = np.float32(1.0 / np.sqrt(HD))
    maskT = np.where(
        np.arange(PB)[None, :] >= np.arange(PB)[:, None], np.float32(0), np.float32(NEG)
    ).astype(np.float32)  # [k, q]: masked where q < k
    vpad = np.zeros((PB, N_KB, VBLK - HD), BF)
    vpad[:, :, 0] = 1

    x2s, cs1s, cs2s = [], [], []
    for b in range(B):
        xb = np.asarray(x[b], np.float32)  # [S, D]
        x2s.append(_tile_p(xb.T, S).astype(BF))
        cosb = np.asarray(freqs_cos[b], np.float32).T  # [64, S]
        sinb = np.asarray(freqs_sin[b], np.float32).T
        cs1s.append(np.concatenate([cosb, sinb], axis=0).astype(BF))
        cs2s.append(np.concatenate([sinb, cosb], axis=0).astype(BF))

    in_maps = []
    for c in range(N_CORES):
        b, g = divmod(c, N_KV_HEADS)
        qcols = np.concatenate([(HC * g + h) * HD + _HALFSPLIT for h in range(HC)])
        wq_c = np.ascontiguousarray(np.asarray(wq, np.float32)[:, qcols]) * scale
        wk_c = np.ascontiguousarray(np.asarray(wk, np.float32)[:, g * HD + _HALFSPLIT])
        wv_c = np.ascontiguousarray(np.asarray(wv, np.float32)[:, g * HD:(g + 1) * HD])
        wo_c = np.ascontiguousarray(
            np.asarray(wo, np.float32)[g * HC * HD:(g + 1) * HC * HD, :]
        )
        in_maps.append(
            {
                "x2": x2s[b],
                "wq2": _tile_p(wq_c, HC * HD).astype(BF),
                "wk2": _tile_p(wk_c, HD).astype(BF),
                "wv2": _tile_p(wv_c, HD).astype(BF),
                "wo2": _tile_p(wo_c, D).astype(BF),
                "cs1": cs1s[b],
                "cs2": cs2s[b],
                "maskT": maskT,
                "vpad": vpad,
            }
        )
    return in_maps


_CACHE = {}


def get_runner(repeat=1, chain=1):
    """Build (once) the Bass module and a cached jitted 8-core executor."""
    if (repeat, chain) in _CACHE:
        return _CACHE[(repeat, chain)]
    import jax
    from jax.sharding import Mesh, PartitionSpec
    from jax.experimental.shard_map import shard_map
    from concourse.bass2jax import (
        _bass_exec_p,
        install_neuronx_cc_hook,
        partition_id_tensor,
    )

    nc = build_nc(repeat=repeat)
    install_neuronx_cc_hook()
    partition_name = nc.partition_id_tensor.name if nc.partition_id_tensor else None
    in_names, out_names, out_avals = [], [], []
    for alloc in nc.m.functions[0].allocations:
        if not isinstance(alloc, mybir.MemoryLocationSet):
            continue
        name = alloc.memorylocations[0].name
        if alloc.kind == "ExternalInput":
            if name != partition_name:
                in_names.append(name)
        elif alloc.kind == "ExternalOutput":
            out_names.append(name)
            out_avals.append(
                jax.core.ShapedArray(tuple(alloc.tensor_shape), mybir.dt.np(alloc.dtype))
            )
    n_params = len(in_names)
    n_outs = len(out_avals)
    all_in_names = list(in_names) + list(out_names)
    if partition_name is not None:
        all_in_names.append(partition_name)

    def _body(*args):
        operands = list(args)
        if partition_name is not None:
            operands.append(partition_id_tensor())
        outs = _bass_exec_p.bind(
            *operands,
            out_avals=tuple(out_avals),
            in_names=tuple(all_in_names),
            out_names=tuple(out_names),
            lowering_input_output_aliases=(),
            sim_require_finite=True,
            sim_require_nnan=True,
            nc=nc,
        )
        return tuple(outs)

    devices = jax.devices()[:N_CORES]
    mesh = Mesh(np.asarray(devices), ("core",))
    in_specs = (PartitionSpec("core"),) * (n_params + n_outs)
    out_specs = (PartitionSpec("core"),) * n_outs

    def _chain(*args):
        ins, outs = args[:n_params], args[n_params:]
        for _ in range(chain):
            outs = _body(*ins, *outs)
        return outs

    fn = jax.jit(
        shard_map(_chain, mesh=mesh, in_specs=in_specs, out_specs=out_specs, check_rep=False),
        keep_unused=True,
    )

    from jax.sharding import NamedSharding

    sh = NamedSharding(mesh, PartitionSpec("core"))

    def prepare(in_maps):
        concat_in = [
            np.concatenate([m[name] for m in in_maps], axis=0) for name in in_names
        ]
        concat_zeros = [
            np.zeros((N_CORES * a.shape[0], *a.shape[1:]), a.dtype) for a in out_avals
        ]
        return [jax.device_put(a, sh) for a in concat_in + concat_zeros]

    def run_dev(dev_args):
        out_arrs = fn(*dev_args)
        jax.block_until_ready(out_arrs)
        return out_arrs

    def run(in_maps):
        out_arrs = run_dev(prepare(in_maps))
        return np.asarray(out_arrs[0]).reshape(N_CORES, S, D)

    run.prepare = prepare
    run.run_dev = run_dev
    run.fn = fn
    _CACHE[(repeat, chain)] = run
    return run


def kernel(x, wq, wk, wv, wo, freqs_cos, freqs_sin):
    in_maps = make_core_inputs(x, wq, wk, wv, wo, freqs_cos, freqs_sin)
    run = get_runner(repeat=1)
    partials = run(in_maps).astype(np.float32)  # [8, S, D]
    out = np.stack(
        [partials[b * N_KV_HEADS:(b + 1) * N_KV_HEADS].sum(axis=0) for b in range(B)]
    )
    return out.astype(np.float32)
